# revision 10
# baseline (speedup 1.0000x reference)
"""DualMaskRoIPool Trainium2 kernel.

Strategy: shard the 64 ROIs across 8 NeuronCores, clustered by union-box row
range (each core only DMAs a row slice of the feature map) and balanced by
estimated compute cost.  ROI coordinates are known when `kernel()` runs, so
each core gets a specialized straight-line Bass/Tile program:

  per ROI: ScalarE copies the union-box window into an SBUF val buffer,
  GPSIMD memsets the dual-mask complement rectangles to 0 (val == feat*mask
  exactly), and VectorE max-reduces the adaptive 7x7 bin grid, either as a
  single multi-dim reduce per (row-run x col-run) of the grid or as a
  two-stage x-then-y pooling, whichever needs fewer cycles.  All-fp32 max
  ops -> bit-exact vs the reference.

The 8 per-core programs are dispatched concurrently to the 8 devices via the
bass2jax PJRT path.
"""

import numpy as np

PH, PW = 7, 7
SCALE = 0.0625
C, H, W = 128, 56, 56
NCORES = 8
NROIS = 64
DMA_CHUNKS = 4


# ----------------------------------------------------------------- geometry

def _zoom(rois):
    """Exact replica of the reference _zoom (fp32 scale, round-half-even)."""
    s = np.round(rois[:, 1:].astype(np.float32) * np.float32(SCALE)).astype(np.int32)
    x1 = np.where(s[:, 0] >= W, W - 1, s[:, 0])
    y1 = np.where(s[:, 1] >= H, H - 1, s[:, 1])
    x2 = np.where(s[:, 2] >= W, W - 1, s[:, 2])
    y2 = np.where(s[:, 3] >= H, H - 1, s[:, 3])
    return x1, y1, x2, y2


def _bin_edges(lo, extent):
    starts = np.array([lo + (i * extent) // PH for i in range(PH)], np.int64)
    ends = np.array([lo + ((i + 1) * extent + PH - 1) // PH for i in range(PH)], np.int64)
    return starts, ends - starts


def _runs(starts, lens):
    """Split the 7 bins into maximal runs with uniform gap and length."""
    runs = []
    i = 0
    while i < PH:
        n = 1
        gap = 1
        while i + n < PH:
            g = int(starts[i + n] - starts[i + n - 1])
            if lens[i + n] != lens[i]:
                break
            if n == 1:
                gap = g
            elif g != gap:
                break
            n += 1
        runs.append((i, n, gap, int(lens[i])))
        i += n
    return runs


def _complement_rects(mask):
    h, w = mask.shape
    rects = []
    r = 0
    while r < h:
        r2 = r
        while r2 + 1 < h and np.array_equal(mask[r2 + 1], mask[r]):
            r2 += 1
        row = mask[r]
        x = 0
        while x < w:
            if not row[x]:
                x2 = x
                while x2 + 1 < w and not row[x2 + 1]:
                    x2 += 1
                rects.append((r, r2 + 1, x, x2 + 1))
                x = x2 + 1
            else:
                x += 1
        r = r2 + 1
    return rects


def _geometry(rois_1, rois_2):
    x1a, y1a, x2a, y2a = _zoom(np.asarray(rois_1))
    x1b, y1b, x2b, y2b = _zoom(np.asarray(rois_2))
    ux1 = np.minimum(x1a, x1b)
    uy1 = np.minimum(y1a, y1b)
    ux2 = np.maximum(x2a, x2b)
    uy2 = np.maximum(y2a, y2b)
    geoms = []
    for b in range(len(ux1)):
        lo_y, hi_y = int(uy1[b]), int(uy2[b])
        lo_x, hi_x = int(ux1[b]), int(ux2[b])
        h = hi_y - lo_y + 1
        w = hi_x - lo_x + 1
        mask = np.zeros((h, w), bool)
        mask[y1a[b] - lo_y:y2a[b] - lo_y + 1, x1a[b] - lo_x:x2a[b] - lo_x + 1] = True
        mask[y1b[b] - lo_y:y2b[b] - lo_y + 1, x1b[b] - lo_x:x2b[b] - lo_x + 1] = True
        rs, hgt = _bin_edges(lo_y, h)
        cs, wdt = _bin_edges(lo_x, w)
        iruns = _runs(rs, hgt)
        jruns = _runs(cs, wdt)
        # cost (DVE cycles): one-stage = grid cells + overhead per run pair;
        # two-stage = x-pass cells + y-pass cells + overhead per run.
        OVH = 105
        one = sum(ni * hgt_ for (_, ni, _, hgt_) in iruns) \
            * sum(nj * wdt_ for (_, nj, _, wdt_) in jruns) \
            + OVH * len(iruns) * len(jruns)
        xcells = h * sum(nj * wdt_ for (_, nj, _, wdt_) in jruns)
        ycells = PW * sum(ni * hgt_ for (_, ni, _, hgt_) in iruns)
        two = xcells + ycells + OVH * (len(iruns) + len(jruns))
        geoms.append(dict(
            uy1=lo_y, uy2=hi_y, ux1=lo_x, ux2=hi_x, h=h, w=w,
            rects=_complement_rects(mask),
            iruns=iruns, jruns=jruns, rs=rs, cs=cs,
            cost=min(one, two) * 1.05 + 350, two_stage=two < one,
        ))
    return geoms


# ------------------------------------------------------------ program build

def _build_core_program(geoms, ylo, nrows):
    import concourse.bacc as bacc
    import concourse.bass as bass
    import concourse.tile as tile
    from concourse import mybir

    f32 = mybir.dt.float32
    nroi = len(geoms)
    nc = bacc.Bacc("TRN2", target_bir_lowering=False, debug=False)
    feat_d = nc.dram_tensor("feat", [C, nrows * W], f32, kind="ExternalInput").ap()
    out_d = nc.dram_tensor("out", [C, nroi * PH * PW], f32, kind="ExternalOutput").ap()

    maxhw = max((g["h"] * g["w"] for g in geoms if g["rects"]), default=64)
    maxth = max((g["h"] for g in geoms if g["two_stage"]), default=1)

    def sub_ap(base, off, dims):
        p0 = list(list(base.ap)[0])
        return bass.AP(base.tensor, base.offset + off, [p0] + [list(d) for d in dims])

    # chunk boundaries (rows, relative to ylo); first chunk small so the
    # first ROI's window lands early
    bounds = sorted({0, nrows} | {
        min(nrows, max(0, (nrows * t) // 8)) for t in (1, 3, 5)})
    chunks = [(r0, r1) for r0, r1 in zip(bounds[:-1], bounds[1:]) if r1 > r0]

    def chunk_of(row):
        for ci, (r0, r1) in enumerate(chunks):
            if r0 <= row < r1:
                return ci
        raise AssertionError(row)

    with tile.TileContext(nc) as tc:
        with tc.tile_pool(name="main", bufs=1) as pool, \
             tc.tile_pool(name="vals", bufs=4) as vpool:
            feat_ts = []
            for ci, (r0, r1) in enumerate(chunks):
                ft = pool.tile([C, (r1 - r0) * W], f32, tag=f"feat{ci}")
                feat_ts.append(ft)
                nc.sync.dma_start(ft[:], feat_d[:, r0 * W:r1 * W])
            o_t = pool.tile([C, nroi * PH * PW], f32)
            for k, g in enumerate(geoms):
                h, w = g["h"], g["w"]
                rs, cs = g["rs"], g["cs"]
                wy0, wy1 = g["uy1"] - ylo, g["uy2"] - ylo + 1  # window rows rel ylo
                c_lo, c_hi = chunk_of(wy0), chunk_of(wy1 - 1)
                if g["rects"] or c_lo != c_hi:
                    vt = vpool.tile([C, maxhw], f32, tag="v")
                    # copy the window, split at chunk boundaries
                    for ci in range(c_lo, c_hi + 1):
                        r0, r1 = chunks[ci]
                        s0, s1 = max(wy0, r0), min(wy1, r1)
                        win = sub_ap(feat_ts[ci][:], (s0 - r0) * W + g["ux1"],
                                     [[W, s1 - s0], [1, w]])
                        nc.scalar.copy(
                            vt[:, (s0 - wy0) * w:(s1 - wy0) * w].rearrange(
                                "p (a b) -> p a b", a=s1 - s0), win)
                    for (r0, r1, c0, c1) in g["rects"]:
                        nc.scalar.memzero(
                            sub_ap(vt[:], r0 * w + c0, [[w, r1 - r0], [1, c1 - c0]]))
                    src, pitch, oy, ox = vt[:], w, g["uy1"], g["ux1"]
                else:
                    r0 = chunks[c_lo][0]
                    src, pitch, oy, ox = feat_ts[c_lo][:], W, ylo + r0, 0
                if g["two_stage"]:
                    tt = vpool.tile([C, maxth * PW], f32, tag="t")
                    for (j0, nj, gj, wdt) in g["jruns"]:
                        in_ap = sub_ap(
                            src, (g["uy1"] - oy) * pitch + (int(cs[j0]) - ox),
                            [[pitch, h], [gj, nj], [1, wdt]])
                        out_ap = sub_ap(tt[:], j0, [[PW, h], [1, nj]])
                        nc.vector.tensor_reduce(
                            out_ap, in_ap,
                            axis=mybir.AxisListType.X, op=mybir.AluOpType.max)
                    for (i0, ni, gi, hgt) in g["iruns"]:
                        in_ap = sub_ap(
                            tt[:], (int(rs[i0]) - g["uy1"]) * PW,
                            [[gi * PW, ni], [1, PW], [PW, hgt]])
                        out_ap = sub_ap(o_t[:], k * PH * PW + i0 * PW,
                                        [[PW, ni], [1, PW]])
                        nc.vector.tensor_reduce(
                            out_ap, in_ap,
                            axis=mybir.AxisListType.X, op=mybir.AluOpType.max)
                else:
                    for (i0, ni, gi, hgt) in g["iruns"]:
                        for (j0, nj, gj, wdt) in g["jruns"]:
                            in_ap = sub_ap(
                                src,
                                (int(rs[i0]) - oy) * pitch + (int(cs[j0]) - ox),
                                [[gi * pitch, ni], [gj, nj], [pitch, hgt], [1, wdt]])
                            out_ap = sub_ap(
                                o_t[:], k * PH * PW + i0 * PW + j0,
                                [[PW, ni], [1, nj]])
                            nc.vector.tensor_reduce(
                                out_ap, in_ap,
                                axis=mybir.AxisListType.XY, op=mybir.AluOpType.max)
            nc.sync.dma_start(out_d[:], o_t[:])
    nc.compile()
    return nc


# ---------------------------------------------------------------- top level

ROW_NS = 240.0  # marginal cost of one extra feature-map row in a core's slice


def _partition_balanced(geoms):
    """Split y-sorted ROIs into 8 contiguous groups minimizing the max of
    (sum of per-roi costs + row-span cost)."""
    order = sorted(range(NROIS), key=lambda b: geoms[b]["uy1"] + geoms[b]["uy2"])
    costs = [geoms[b]["cost"] for b in order]
    pre = np.concatenate([[0], np.cumsum(costs)])
    n = NROIS
    # span of rows needed by order[i:j]
    lo = np.array([geoms[b]["uy1"] for b in order])
    hi = np.array([geoms[b]["uy2"] for b in order])

    def group_cost(i, j):
        span = hi[i:j].max() - lo[i:j].min() + 1
        return pre[j] - pre[i] + ROW_NS * span

    INF = float("inf")
    dp = np.full((NCORES + 1, n + 1), INF)
    cut = np.zeros((NCORES + 1, n + 1), np.int64)
    dp[0, 0] = 0.0
    for gidx in range(1, NCORES + 1):
        for j in range(gidx, n + 1):
            best, barg = INF, gidx - 1
            for i in range(gidx - 1, j):
                v = max(dp[gidx - 1, i], group_cost(i, j))
                if v < best:
                    best, barg = v, i
            dp[gidx, j] = best
            cut[gidx, j] = barg
    cuts = [n]
    j = n
    for gidx in range(NCORES, 0, -1):
        j = int(cut[gidx, j])
        cuts.append(j)
    cuts = cuts[::-1]
    return [order[cuts[c]:cuts[c + 1]] for c in range(NCORES)]


def _prepare(feature_map, rois_1, rois_2):
    geoms = _geometry(rois_1, rois_2)
    groups = _partition_balanced(geoms)
    fm = np.ascontiguousarray(np.asarray(feature_map), np.float32)[0]  # [C,H,W]
    programs, in_maps, core_ids = [], [], []
    for c in range(NCORES):
        ids = sorted(groups[c], key=lambda b: geoms[b]["uy2"])
        core_geoms = [geoms[b] for b in ids]
        ylo = min(g["uy1"] for g in core_geoms)
        yhi = max(g["uy2"] for g in core_geoms) + 1
        nrows = yhi - ylo
        programs.append(_build_core_program(core_geoms, ylo, nrows))
        in_maps.append({"feat": np.ascontiguousarray(
            fm[:, ylo:yhi, :]).reshape(C, nrows * W)})
        core_ids.append(ids)
    return programs, in_maps, core_ids


def _assemble(outs, core_ids):
    full = np.empty((NROIS, C, PH, PW), np.float32)
    for c in range(NCORES):
        nroi = len(core_ids[c])
        r = outs[c]["out"].reshape(C, nroi, PH, PW).transpose(1, 0, 2, 3)
        for k, b in enumerate(core_ids[c]):
            full[b] = r[k]
    return full


def kernel(feature_map, rois_1, rois_2):
    import jax
    from concourse import bass2jax
    from concurrent.futures import ThreadPoolExecutor

    programs, in_maps, core_ids = _prepare(feature_map, rois_1, rois_2)
    bass2jax.install_neuronx_cc_hook()
    devices = jax.devices()

    def run_one(c):
        with jax.default_device(devices[c]):
            return bass2jax.run_bass_via_pjrt(programs[c], [in_maps[c]], n_cores=1)[0]

    with ThreadPoolExecutor(NCORES) as ex:
        outs = list(ex.map(run_one, range(NCORES)))
    return _assemble(outs, core_ids)


# revision 12
# speedup vs baseline: 1.1868x; 1.1868x over previous
"""DualMaskRoIPool Trainium2 kernel.

Strategy: shard the 64 ROIs across 8 NeuronCores, clustered by union-box row
range (each core only DMAs a row slice of the feature map) and balanced by a
calibrated per-ROI cost model.  ROI coordinates are known when `kernel()`
runs, so each core gets a specialized straight-line Bass/Tile program:

  per ROI: ScalarE copies the union-box window into an SBUF val buffer,
  GPSIMD memsets the dual-mask complement rectangles to 0 (val == feat*mask
  exactly), and VectorE max-reduces the adaptive 7x7 bin grid — either one
  multi-dim reduce per (row-run x col-run) of the grid, or two-stage
  x-then-y pooling with all-zero row bands / all-zero bins skipped (their
  x-pool result is memset to 0 instead).  All-fp32 max ops -> bit-exact vs
  the reference.

The 8 per-core programs are dispatched concurrently to the 8 devices via the
bass2jax PJRT path.
"""

import numpy as np

PH, PW = 7, 7
SCALE = 0.0625
C, H, W = 128, 56, 56
NCORES = 8
NROIS = 64

OVH = 105       # DVE per-instruction overhead, cycles ~ ns
MEMSET_NS = 60  # GPSIMD memset cost (hidden, but count a little)
ROW_NS = 240.0  # marginal cost of one extra feature-map row in a core slice


# ----------------------------------------------------------------- geometry

def _zoom(rois):
    """Exact replica of the reference _zoom (fp32 scale, round-half-even)."""
    s = np.round(rois[:, 1:].astype(np.float32) * np.float32(SCALE)).astype(np.int32)
    x1 = np.where(s[:, 0] >= W, W - 1, s[:, 0])
    y1 = np.where(s[:, 1] >= H, H - 1, s[:, 1])
    x2 = np.where(s[:, 2] >= W, W - 1, s[:, 2])
    y2 = np.where(s[:, 3] >= H, H - 1, s[:, 3])
    return x1, y1, x2, y2


def _bin_edges(lo, extent):
    starts = np.array([lo + (i * extent) // PH for i in range(PH)], np.int64)
    ends = np.array([lo + ((i + 1) * extent + PH - 1) // PH for i in range(PH)], np.int64)
    return starts, ends - starts


def _runs_idx(idxs, starts, lens):
    """Maximal uniform-gap/uniform-length runs over the given bin indices."""
    runs = []
    i = 0
    n = len(idxs)
    while i < n:
        cnt = 1
        gap = 1
        while i + cnt < n:
            if idxs[i + cnt] != idxs[i + cnt - 1] + 1:
                break
            g = int(starts[idxs[i + cnt]] - starts[idxs[i + cnt - 1]])
            if lens[idxs[i + cnt]] != lens[idxs[i]]:
                break
            if cnt == 1:
                gap = g
            elif g != gap:
                break
            cnt += 1
        runs.append((idxs[i], cnt, gap, int(lens[idxs[i]])))
        i += cnt
    return runs


def _runs(starts, lens):
    return _runs_idx(list(range(PH)), starts, lens)


def _complement_rects(mask):
    h, w = mask.shape
    rects = []
    r = 0
    while r < h:
        r2 = r
        while r2 + 1 < h and np.array_equal(mask[r2 + 1], mask[r]):
            r2 += 1
        row = mask[r]
        x = 0
        while x < w:
            if not row[x]:
                x2 = x
                while x2 + 1 < w and not row[x2 + 1]:
                    x2 += 1
                rects.append((r, r2 + 1, x, x2 + 1))
                x = x2 + 1
            else:
                x += 1
        r = r2 + 1
    return rects


def _plan_roi(mask, rs, hgt, cs, wdt, uy1, ux1):
    """Build the instruction plan for one ROI.  Returns a dict with:
    one_stage: bool; for two-stage: xinstrs [(row0, nrow, jrun)], tmemsets
    [(start_elem, nelem, stride)], yruns; cost estimate."""
    h, w = mask.shape
    iruns = _runs(rs, hgt)
    jruns = _runs(cs, wdt)
    sj = sum(nj * wd for (_, nj, _, wd) in jruns)
    si = sum(ni * hg for (_, ni, _, hg) in iruns)

    one_cost = si * sj + OVH * len(iruns) * len(jruns)

    # --- two-stage with zero-row / zero-bin clipping ---
    nzrow = mask.any(axis=1)
    segments = []  # (r0, r1) of contiguous non-zero rows
    r = 0
    while r < h:
        if nzrow[r]:
            r2 = r
            while r2 + 1 < h and nzrow[r2 + 1]:
                r2 += 1
            segments.append((r, r2 + 1))
            r = r2 + 1
        else:
            r += 1
    # fall back to a single full segment if splitting isn't worth it
    nzero_rows = h - int(nzrow.sum())
    if len(segments) > 1 and nzero_rows * sj < (len(segments) - 1) * len(jruns) * OVH:
        segments = [(0, h)]
    if not segments:
        segments = [(0, h)]

    xinstrs = []   # (r0, nrow, [runs over kept bins])
    tmemsets = []  # (elem_offset, [dims]) for tmp regions forced to 0
    xcells = 0
    nxi = 0
    prev_end = 0
    for (a, b) in segments:
        if a > prev_end:
            tmemsets.append((prev_end * PW, [[1, (a - prev_end) * PW]]))
        prev_end = b
        seg_zero_col = ~mask[a:b].any(axis=0)
        kept = []
        for j in range(PW):
            c0 = int(cs[j]) - ux1
            wd = int(wdt[j])
            if seg_zero_col[c0:c0 + wd].all() and (b - a) * wd > 250:
                tmemsets.append((a * PW + j, [[PW, b - a]]))
            else:
                kept.append(j)
        runs = _runs_idx(kept, cs, wdt)
        xinstrs.append((a, b - a, runs))
        nxi += len(runs)
        xcells += (b - a) * sum(nj * wd for (_, nj, _, wd) in runs)
    if prev_end < h:
        tmemsets.append((prev_end * PW, [[1, (h - prev_end) * PW]]))

    two_cost = xcells + PW * si + OVH * (nxi + len(iruns)) \
        + MEMSET_NS * len(tmemsets)

    one = one_cost <= two_cost
    return dict(
        one_stage=one,
        iruns=iruns, jruns=jruns,
        xinstrs=xinstrs, tmemsets=tmemsets,
        cost=min(one_cost, two_cost) * 1.05 + 350,
    )


def _geometry(rois_1, rois_2):
    x1a, y1a, x2a, y2a = _zoom(np.asarray(rois_1))
    x1b, y1b, x2b, y2b = _zoom(np.asarray(rois_2))
    ux1 = np.minimum(x1a, x1b)
    uy1 = np.minimum(y1a, y1b)
    ux2 = np.maximum(x2a, x2b)
    uy2 = np.maximum(y2a, y2b)
    geoms = []
    for b in range(len(ux1)):
        lo_y, hi_y = int(uy1[b]), int(uy2[b])
        lo_x, hi_x = int(ux1[b]), int(ux2[b])
        h = hi_y - lo_y + 1
        w = hi_x - lo_x + 1
        mask = np.zeros((h, w), bool)
        mask[y1a[b] - lo_y:y2a[b] - lo_y + 1, x1a[b] - lo_x:x2a[b] - lo_x + 1] = True
        mask[y1b[b] - lo_y:y2b[b] - lo_y + 1, x1b[b] - lo_x:x2b[b] - lo_x + 1] = True
        rs, hgt = _bin_edges(lo_y, h)
        cs, wdt = _bin_edges(lo_x, w)
        g = dict(
            uy1=lo_y, uy2=hi_y, ux1=lo_x, ux2=hi_x, h=h, w=w,
            rects=_complement_rects(mask), rs=rs, cs=cs,
        )
        g.update(_plan_roi(mask, rs, hgt, cs, wdt, lo_y, lo_x))
        geoms.append(g)
    return geoms


# ------------------------------------------------------------ program build

def _build_core_program(geoms, ylo, nrows):
    import concourse.bacc as bacc
    import concourse.bass as bass
    import concourse.tile as tile
    from concourse import mybir

    f32 = mybir.dt.float32
    nroi = len(geoms)
    nc = bacc.Bacc("TRN2", target_bir_lowering=False, debug=False)
    feat_d = nc.dram_tensor("feat", [C, nrows * W], f32, kind="ExternalInput").ap()
    out_d = nc.dram_tensor("out", [C, nroi * PH * PW], f32, kind="ExternalOutput").ap()

    maxhw = max((g["h"] * g["w"] for g in geoms), default=64)
    maxth = max((g["h"] for g in geoms if not g["one_stage"]), default=1)

    def sub_ap(base, off, dims):
        p0 = list(list(base.ap)[0])
        return bass.AP(base.tensor, base.offset + off, [p0] + [list(d) for d in dims])

    # chunk boundaries (rows, relative to ylo); small first chunk
    bounds = sorted({0, nrows} | {
        min(nrows, max(0, (nrows * t) // 8)) for t in (1, 3, 5)})
    chunks = [(r0, r1) for r0, r1 in zip(bounds[:-1], bounds[1:]) if r1 > r0]

    def chunk_of(row):
        for ci, (r0, r1) in enumerate(chunks):
            if r0 <= row < r1:
                return ci
        raise AssertionError(row)

    with tile.TileContext(nc) as tc:
        with tc.tile_pool(name="main", bufs=1) as pool, \
             tc.tile_pool(name="vals", bufs=4) as vpool:
            feat_ts = []
            for ci, (r0, r1) in enumerate(chunks):
                ft = pool.tile([C, (r1 - r0) * W], f32, tag=f"feat{ci}")
                feat_ts.append(ft)
                nc.sync.dma_start(ft[:], feat_d[:, r0 * W:r1 * W])
            o_t = pool.tile([C, nroi * PH * PW], f32)
            for k, g in enumerate(geoms):
                h, w = g["h"], g["w"]
                rs, cs = g["rs"], g["cs"]
                wy0, wy1 = g["uy1"] - ylo, g["uy2"] - ylo + 1
                c_lo, c_hi = chunk_of(wy0), chunk_of(wy1 - 1)
                if g["rects"] or c_lo != c_hi:
                    vt = vpool.tile([C, maxhw], f32, tag="v")
                    for ci in range(c_lo, c_hi + 1):
                        r0, r1 = chunks[ci]
                        s0, s1 = max(wy0, r0), min(wy1, r1)
                        win = sub_ap(feat_ts[ci][:], (s0 - r0) * W + g["ux1"],
                                     [[W, s1 - s0], [1, w]])
                        nc.scalar.copy(
                            vt[:, (s0 - wy0) * w:(s1 - wy0) * w].rearrange(
                                "p (a b) -> p a b", a=s1 - s0), win)
                    for (r0, r1, c0, c1) in g["rects"]:
                        nc.gpsimd.memset(
                            sub_ap(vt[:], r0 * w + c0, [[w, r1 - r0], [1, c1 - c0]]),
                            0.0)
                    src, pitch, oy, ox = vt[:], w, g["uy1"], g["ux1"]
                else:
                    r0 = chunks[c_lo][0]
                    src, pitch, oy, ox = feat_ts[c_lo][:], W, ylo + r0, 0
                if not g["one_stage"]:
                    tt = vpool.tile([C, maxth * PW], f32, tag="t")
                    for (off, dims) in g["tmemsets"]:
                        nc.gpsimd.memset(sub_ap(tt[:], off, dims), 0.0)
                    for (a, nrow, runs) in g["xinstrs"]:
                        for (j0, nj, gj, wdt) in runs:
                            in_ap = sub_ap(
                                src,
                                (g["uy1"] + a - oy) * pitch + (int(cs[j0]) - ox),
                                [[pitch, nrow], [gj, nj], [1, wdt]])
                            out_ap = sub_ap(tt[:], a * PW + j0,
                                            [[PW, nrow], [1, nj]])
                            nc.vector.tensor_reduce(
                                out_ap, in_ap,
                                axis=mybir.AxisListType.X, op=mybir.AluOpType.max)
                    for (i0, ni, gi, hgt) in g["iruns"]:
                        in_ap = sub_ap(
                            tt[:], (int(rs[i0]) - g["uy1"]) * PW,
                            [[gi * PW, ni], [1, PW], [PW, hgt]])
                        out_ap = sub_ap(o_t[:], k * PH * PW + i0 * PW,
                                        [[PW, ni], [1, PW]])
                        nc.vector.tensor_reduce(
                            out_ap, in_ap,
                            axis=mybir.AxisListType.X, op=mybir.AluOpType.max)
                else:
                    for (i0, ni, gi, hgt) in g["iruns"]:
                        for (j0, nj, gj, wdt) in g["jruns"]:
                            in_ap = sub_ap(
                                src,
                                (int(rs[i0]) - oy) * pitch + (int(cs[j0]) - ox),
                                [[gi * pitch, ni], [gj, nj], [pitch, hgt], [1, wdt]])
                            out_ap = sub_ap(
                                o_t[:], k * PH * PW + i0 * PW + j0,
                                [[PW, ni], [1, nj]])
                            nc.vector.tensor_reduce(
                                out_ap, in_ap,
                                axis=mybir.AxisListType.XY, op=mybir.AluOpType.max)
            nc.sync.dma_start(out_d[:], o_t[:])
    nc.compile()
    return nc


# ---------------------------------------------------------------- top level

def _partition_balanced(geoms):
    """Split y-sorted ROIs into 8 contiguous groups minimizing the max of
    (sum of per-roi costs + row-span cost)."""
    order = sorted(range(NROIS), key=lambda b: geoms[b]["uy1"] + geoms[b]["uy2"])
    costs = [geoms[b]["cost"] for b in order]
    pre = np.concatenate([[0], np.cumsum(costs)])
    n = NROIS
    lo = np.array([geoms[b]["uy1"] for b in order])
    hi = np.array([geoms[b]["uy2"] for b in order])

    def group_cost(i, j):
        span = hi[i:j].max() - lo[i:j].min() + 1
        return pre[j] - pre[i] + ROW_NS * span

    INF = float("inf")
    dp = np.full((NCORES + 1, n + 1), INF)
    cut = np.zeros((NCORES + 1, n + 1), np.int64)
    dp[0, 0] = 0.0
    for gidx in range(1, NCORES + 1):
        for j in range(gidx, n + 1):
            best, barg = INF, gidx - 1
            for i in range(gidx - 1, j):
                v = max(dp[gidx - 1, i], group_cost(i, j))
                if v < best:
                    best, barg = v, i
            dp[gidx, j] = best
            cut[gidx, j] = barg
    cuts = [n]
    j = n
    for gidx in range(NCORES, 0, -1):
        j = int(cut[gidx, j])
        cuts.append(j)
    cuts = cuts[::-1]
    return [order[cuts[c]:cuts[c + 1]] for c in range(NCORES)]


def _prepare(feature_map, rois_1, rois_2):
    geoms = _geometry(rois_1, rois_2)
    groups = _partition_balanced(geoms)
    fm = np.ascontiguousarray(np.asarray(feature_map), np.float32)[0]  # [C,H,W]
    programs, in_maps, core_ids = [], [], []
    for c in range(NCORES):
        ids = sorted(groups[c], key=lambda b: geoms[b]["uy2"])
        # lead with a no-copy ROI if one sits near the front (starts sooner)
        for t in range(min(3, len(ids))):
            if not geoms[ids[t]]["rects"]:
                ids.insert(0, ids.pop(t))
                break
        core_geoms = [geoms[b] for b in ids]
        ylo = min(g["uy1"] for g in core_geoms)
        yhi = max(g["uy2"] for g in core_geoms) + 1
        nrows = yhi - ylo
        programs.append(_build_core_program(core_geoms, ylo, nrows))
        in_maps.append({"feat": np.ascontiguousarray(
            fm[:, ylo:yhi, :]).reshape(C, nrows * W)})
        core_ids.append(ids)
    return programs, in_maps, core_ids


def _assemble(outs, core_ids):
    full = np.empty((NROIS, C, PH, PW), np.float32)
    for c in range(NCORES):
        nroi = len(core_ids[c])
        r = outs[c]["out"].reshape(C, nroi, PH, PW).transpose(1, 0, 2, 3)
        for k, b in enumerate(core_ids[c]):
            full[b] = r[k]
    return full


def kernel(feature_map, rois_1, rois_2):
    import jax
    from concourse import bass2jax
    from concurrent.futures import ThreadPoolExecutor

    programs, in_maps, core_ids = _prepare(feature_map, rois_1, rois_2)
    bass2jax.install_neuronx_cc_hook()
    devices = jax.devices()

    def run_one(c):
        with jax.default_device(devices[c]):
            return bass2jax.run_bass_via_pjrt(programs[c], [in_maps[c]], n_cores=1)[0]

    with ThreadPoolExecutor(NCORES) as ex:
        outs = list(ex.map(run_one, range(NCORES)))
    return _assemble(outs, core_ids)
